# revision 2
# baseline (speedup 1.0000x reference)
"""AnchorLoss Trainium2 kernel.

loss = sum_{b,i,j: mask[b,i,j]==1} (1 - exp(-|z_i - z_j|^2 / 10)),  z = embedding + abs_coords

Sharding: data-parallel over batch B=8, one batch per NeuronCore. Each core:
  - computes zT/rT prep from its [2048, 2] embedding/abs_coords slice,
  - streams its [2048, 2048] int32 mask in 16 row-blocks of [128, 2048],
  - per block: one K=4 f32 matmul -> PSUM holds -d2/10, ScalarE exp in place,
    VectorE fused (E - 1) * mask with per-partition accumulate,
  - returns [128, 16] partial sums; host sums and negates.
"""
import numpy as np
import sys

for _p in ("/opt/trn_rl_repo", "/root/.axon_site/_ro/trn_rl_repo"):
    if _p not in sys.path:
        sys.path.append(_p)

N = 2048
NB = N // 128  # 16 row blocks
NJ = N // 512  # 4 matmul column chunks
B = 8

_CACHED = None


def _build(n=N):
    from concourse import bacc, mybir, tile

    f32 = mybir.dt.float32
    i32 = mybir.dt.int32
    AF = mybir.ActivationFunctionType
    ALU = mybir.AluOpType

    nb = n // 128
    nj = max(n // 512, 1)
    jw = min(n, 512)

    nc = bacc.Bacc()
    e_in = nc.declare_dram_parameter("e", [n, 2], f32, isOutput=False)
    a_in = nc.declare_dram_parameter("a", [n, 2], f32, isOutput=False)
    m_in = nc.declare_dram_parameter("m", [n, n], i32, isOutput=False)
    out = nc.declare_dram_parameter("out", [128, nb], f32, isOutput=True)

    with tile.TileContext(nc) as tc:
        with (
            tc.tile_pool(name="singles", bufs=1) as singles,
            tc.tile_pool(name="maskp", bufs=3) as maskp,
            tc.tile_pool(name="psum", bufs=2, space="PSUM") as psump,
        ):
            # ---- prep: zT [2, n] and rT [1, n] ----
            et = singles.tile([2, n], f32)
            at = singles.tile([2, n], f32)
            nc.sync.dma_start(et[:], e_in.rearrange("n d -> d n"))
            nc.sync.dma_start(at[:], a_in.rearrange("n d -> d n"))
            zt = singles.tile([2, n], f32)
            nc.vector.tensor_tensor(zt[:], et[:], at[:], ALU.add)
            sq = singles.tile([2, n], f32)
            nc.vector.tensor_tensor(sq[:], zt[:], zt[:], ALU.mult)
            rrow = singles.tile([1, n], f32)
            nc.gpsimd.dma_start(rrow[:], sq[0:1, :])
            nc.gpsimd.dma_start(rrow[:], sq[1:2, :], accum_op=ALU.add)

            # compute engines only accept start partitions {0,32,64,96}:
            # compute rows at base 0, place them with SBUF->SBUF DMA.
            ones1 = singles.tile([1, n], f32)
            nc.vector.memset(ones1[:], 1.0)
            mr = singles.tile([1, n], f32)  # -0.1 * r
            nc.vector.tensor_scalar_mul(mr[:], rrow[:], -0.1)
            zs = singles.tile([2, n], f32)  # 0.2 * z
            nc.vector.tensor_scalar_mul(zs[:], zt[:], 0.2)

            # rhs rows (j side): [-0.1 r_j, zx_j, zy_j, 1]
            zcol = singles.tile([4, n], f32)
            nc.gpsimd.dma_start(zcol[0:1, :], mr[:])
            nc.gpsimd.dma_start(zcol[1:3, :], zt[:])
            nc.gpsimd.dma_start(zcol[3:4, :], ones1[:])

            # lhsT rows (i side): [1, 0.2 zx_i, 0.2 zy_i, -0.1 r_i]
            zrow = singles.tile([4, n], f32)
            nc.gpsimd.dma_start(zrow[0:1, :], ones1[:])
            nc.gpsimd.dma_start(zrow[1:3, :], zs[:])
            nc.gpsimd.dma_start(zrow[3:4, :], mr[:])

            acc = singles.tile([128, nb], f32)

            # ---- main loop over row blocks ----
            for ib in range(nb):
                mk = maskp.tile([128, n], i32)
                nc.sync.dma_start(mk[:], m_in[ib * 128:(ib + 1) * 128, :])
                ps = psump.tile([128, n], f32)
                for jc in range(nj):
                    nc.tensor.matmul(
                        ps[:, jc * jw:(jc + 1) * jw],
                        zrow[:, ib * 128:(ib + 1) * 128],
                        zcol[:, jc * jw:(jc + 1) * jw],
                        start=True,
                        stop=True,
                    )
                nc.scalar.activation(ps[:], ps[:], AF.Exp)
                nc.vector.scalar_tensor_tensor(
                    ps[:], ps[:], 1.0, mk[:],
                    op0=ALU.subtract, op1=ALU.mult,
                    accum_out=acc[:, ib:ib + 1],
                )
            nc.sync.dma_start(out[:], acc[:])
    nc.compile()
    return nc


def _get_graph():
    global _CACHED
    if _CACHED is None:
        _CACHED = _build()
    return _CACHED


def kernel(embedding, abs_coords, patch_mask, _trace=False, _trace_kwargs=None):
    from concourse.bass_utils import run_bass_kernel_spmd

    nc = _get_graph()
    in_maps = [
        {
            "e": np.ascontiguousarray(embedding[b], dtype=np.float32),
            "a": np.ascontiguousarray(abs_coords[b], dtype=np.float32),
            "m": np.ascontiguousarray(patch_mask[b], dtype=np.int32),
        }
        for b in range(B)
    ]
    kw = {}
    if _trace:
        kw = dict(trace=True, **(_trace_kwargs or {}))
    res = run_bass_kernel_spmd(nc, in_maps, core_ids=list(range(B)), **kw)
    total = -sum(
        float(np.sum(r["out"], dtype=np.float64)) for r in res.results
    )
    out = np.float32(total)
    if _trace:
        return out, res
    return out


# revision 4
# speedup vs baseline: 1.4120x; 1.4120x over previous
"""AnchorLoss Trainium2 kernel.

loss = sum_{b,i,j: mask[b,i,j]==1} (1 - exp(-|z_i - z_j|^2 / 10)),  z = embedding + abs_coords

Sharding: data-parallel over batch B=8, one batch per NeuronCore. Each core:
  - computes zT/rT prep from its [2048, 2] embedding/abs_coords slice,
  - streams its [2048, 2048] int32 mask in 16 row-blocks of [128, 2048],
  - per block: one K=4 f32 matmul -> PSUM holds -d2/10, ScalarE exp in place,
    VectorE fused (E - 1) * mask with per-partition accumulate,
  - returns [128, 16] partial sums; host sums and negates.
"""
import numpy as np
import sys

for _p in ("/opt/trn_rl_repo", "/root/.axon_site/_ro/trn_rl_repo"):
    if _p not in sys.path:
        sys.path.append(_p)

N = 2048
NB = N // 128  # 16 row blocks
NJ = N // 512  # 4 matmul column chunks
B = 8

_CACHED = None


def _build(n=N):
    from concourse import bacc, mybir, tile

    f32 = mybir.dt.float32
    i32 = mybir.dt.int32
    AF = mybir.ActivationFunctionType
    ALU = mybir.AluOpType

    bf16 = mybir.dt.bfloat16
    nb = n // 128
    nj = max(n // 512, 1)
    jw = min(n, 512)

    nc = bacc.Bacc()
    e_in = nc.declare_dram_parameter("e", [n, 2], f32, isOutput=False)
    a_in = nc.declare_dram_parameter("a", [n, 2], f32, isOutput=False)
    m_in = nc.declare_dram_parameter("m", [n, n], i32, isOutput=False)
    out = nc.declare_dram_parameter("out", [128, nb], f32, isOutput=True)

    with tile.TileContext(nc) as tc:
        with (
            tc.tile_pool(name="singles", bufs=1) as singles,
            tc.tile_pool(name="maskp", bufs=3) as maskp,
            tc.tile_pool(name="psum", bufs=2, space="PSUM") as psump,
        ):
            # ---- prep: zT [2, n] and rT [1, n] ----
            et = singles.tile([2, n], f32)
            at = singles.tile([2, n], f32)
            nc.sync.dma_start(et[:], e_in.rearrange("n d -> d n"))
            nc.sync.dma_start(at[:], a_in.rearrange("n d -> d n"))
            zt = singles.tile([2, n], f32)
            nc.vector.tensor_tensor(zt[:], et[:], at[:], ALU.add)
            sq = singles.tile([2, n], f32)
            nc.vector.tensor_tensor(sq[:], zt[:], zt[:], ALU.mult)
            rrow = singles.tile([1, n], f32)
            nc.gpsimd.dma_start(rrow[:], sq[0:1, :])
            nc.gpsimd.dma_start(rrow[:], sq[1:2, :], accum_op=ALU.add)

            # ---- bf16 hi/lo split (pseudo-fp32 matmul operands) ----
            # Sources: z (j side), s = 0.2 z (i side), mr = -0.1 r (both).
            # hi = bf16(v); lo = bf16(v - hi). Products keep ~2^-17 rel acc.
            # Compute engines only accept start partitions {0,32,64,96}:
            # everything is computed at base 0, placed with SBUF->SBUF DMA.
            zh = singles.tile([2, n], bf16)
            zl = singles.tile([2, n], bf16)
            sh = singles.tile([2, n], bf16)
            sl = singles.tile([2, n], bf16)
            mrh = singles.tile([1, n], bf16)
            mrl = singles.tile([1, n], bf16)
            tmp = singles.tile([2, n], f32)

            # z: hi on ACT, lo residual on DVE, round on DVE
            nc.scalar.activation(zh[:], zt[:], AF.Copy)
            nc.vector.tensor_tensor(tmp[:], zt[:], zh[:], ALU.subtract)
            nc.vector.tensor_copy(zl[:], tmp[:])
            # s = 0.2 z
            nc.scalar.activation(sh[:], zt[:], AF.Copy, scale=0.2)
            nc.vector.scalar_tensor_tensor(
                tmp[:], zt[:], 0.2, sh[:], op0=ALU.mult, op1=ALU.subtract
            )
            nc.vector.tensor_copy(sl[:], tmp[:])
            # mr = -0.1 r
            nc.scalar.activation(mrh[:], rrow[:], AF.Copy, scale=-0.1)
            nc.vector.scalar_tensor_tensor(
                tmp[0:1, :], rrow[:], -0.1, mrh[:], op0=ALU.mult, op1=ALU.subtract
            )
            nc.vector.tensor_copy(mrl[:], tmp[0:1, :])

            ones2 = singles.tile([2, n], bf16)
            nc.vector.memset(ones2[:], 1.0)

            # K=10 row pairing (lhsT row k x rhs row k):
            #  k0: 1      * mrh_j   k1: 1      * mrl_j
            #  k2: mrh_i  * 1       k3: mrl_i  * 1
            #  k4: sxh_i  * zxh_j   k5: syh_i  * zyh_j
            #  k6: sxh_i  * zxl_j   k7: syh_i  * zyl_j
            #  k8: sxl_i  * zxh_j   k9: syl_i  * zyh_j
            zcol = singles.tile([10, n], bf16)  # rhs (j side)
            nc.gpsimd.dma_start(zcol[0:1, :], mrh[:])
            nc.gpsimd.dma_start(zcol[1:2, :], mrl[:])
            nc.gpsimd.dma_start(zcol[2:4, :], ones2[:])
            nc.gpsimd.dma_start(zcol[4:6, :], zh[:])
            nc.gpsimd.dma_start(zcol[6:8, :], zl[:])
            nc.gpsimd.dma_start(zcol[8:10, :], zh[:])

            zrow = singles.tile([10, n], bf16)  # lhsT (i side)
            nc.gpsimd.dma_start(zrow[0:2, :], ones2[:])
            nc.gpsimd.dma_start(zrow[2:3, :], mrh[:])
            nc.gpsimd.dma_start(zrow[3:4, :], mrl[:])
            nc.gpsimd.dma_start(zrow[4:6, :], sh[:])
            nc.gpsimd.dma_start(zrow[6:8, :], sh[:])
            nc.gpsimd.dma_start(zrow[8:10, :], sl[:])

            acc = singles.tile([128, nb], f32)

            # ---- main loop over row blocks ----
            for ib in range(nb):
                mk = maskp.tile([128, n], i32)
                nc.sync.dma_start(mk[:], m_in[ib * 128:(ib + 1) * 128, :])
                ps = psump.tile([128, n], f32)
                for jc in range(nj):
                    nc.tensor.matmul(
                        ps[:, jc * jw:(jc + 1) * jw],
                        zrow[:, ib * 128:(ib + 1) * 128],
                        zcol[:, jc * jw:(jc + 1) * jw],
                        start=True,
                        stop=True,
                    )
                nc.scalar.activation(ps[:], ps[:], AF.Exp)
                nc.vector.scalar_tensor_tensor(
                    ps[:], ps[:], 1.0, mk[:],
                    op0=ALU.subtract, op1=ALU.mult,
                    accum_out=acc[:, ib:ib + 1],
                )
            nc.sync.dma_start(out[:], acc[:])
    nc.compile()
    return nc


def _get_graph():
    global _CACHED
    if _CACHED is None:
        _CACHED = _build()
    return _CACHED


def kernel(embedding, abs_coords, patch_mask, _trace=False, _trace_kwargs=None):
    from concourse.bass_utils import run_bass_kernel_spmd

    nc = _get_graph()
    in_maps = [
        {
            "e": np.ascontiguousarray(embedding[b], dtype=np.float32),
            "a": np.ascontiguousarray(abs_coords[b], dtype=np.float32),
            "m": np.ascontiguousarray(patch_mask[b], dtype=np.int32),
        }
        for b in range(B)
    ]
    kw = {}
    if _trace:
        kw = dict(trace=True, **(_trace_kwargs or {}))
    res = run_bass_kernel_spmd(nc, in_maps, core_ids=list(range(B)), **kw)
    total = -sum(
        float(np.sum(r["out"], dtype=np.float64)) for r in res.results
    )
    out = np.float32(total)
    if _trace:
        return out, res
    return out


# revision 5
# speedup vs baseline: 1.8927x; 1.3405x over previous
"""AnchorLoss Trainium2 kernel.

loss = sum_{b,i,j: mask[b,i,j]==1} (1 - exp(-|z_i - z_j|^2 / 10)),  z = embedding + abs_coords

Sharding: data-parallel over batch B=8, one batch per NeuronCore. Each core:
  - device-side prep: z = e + a, r = |z|^2, bf16 hi/lo splits (pseudo-fp32),
  - streams its [2048, 2048] int32 mask in 16 row-blocks of [128, 2048],
  - per 1024-col chunk: K=10 bf16 matmul -> PSUM = -d2/10, ScalarE exp in
    place, VectorE fused (E - 1) * mask with per-partition accumulate,
  - returns [128, 32] partial sums; host sums and negates.

The host passes e/a pre-transposed [2, N] (layout only, zero flops) so the
coordinate loads are 2-descriptor DMAs instead of 4096-descriptor ones.
"""
import numpy as np
import sys

for _p in ("/opt/trn_rl_repo", "/root/.axon_site/_ro/trn_rl_repo"):
    if _p not in sys.path:
        sys.path.append(_p)

N = 2048
B = 8

_CACHED = None


def _build(n=N):
    from concourse import bacc, mybir, tile

    f32 = mybir.dt.float32
    i32 = mybir.dt.int32
    bf16 = mybir.dt.bfloat16
    AF = mybir.ActivationFunctionType
    ALU = mybir.AluOpType

    nb = n // 128          # mask row blocks
    cw = min(n, 1024)      # pipeline chunk width
    nch = n // cw          # chunks per row block
    nj = cw // 512         # matmuls per chunk

    nc = bacc.Bacc()
    e_in = nc.declare_dram_parameter("e", [2, n], f32, isOutput=False)
    a_in = nc.declare_dram_parameter("a", [2, n], f32, isOutput=False)
    m_in = nc.declare_dram_parameter("m", [n, n], i32, isOutput=False)
    out = nc.declare_dram_parameter("out", [128, nb * nch], f32, isOutput=True)

    with tile.TileContext(nc) as tc:
        with (
            tc.tile_pool(name="singles", bufs=1) as singles,
            tc.tile_pool(name="maskp", bufs=3) as maskp,
            tc.tile_pool(name="psum", bufs=4, space="PSUM") as psump,
        ):
            # ---- prep: zT [2, n], rT [1, n] ----
            et = singles.tile([2, n], f32)
            at = singles.tile([2, n], f32)
            nc.gpsimd.dma_start(et[:], e_in[:])
            nc.gpsimd.dma_start(at[:], a_in[:])
            zt = singles.tile([2, n], f32)
            nc.vector.tensor_tensor(zt[:], et[:], at[:], ALU.add)
            sq = singles.tile([2, n], f32)
            nc.vector.tensor_tensor(sq[:], zt[:], zt[:], ALU.mult)
            rrow = singles.tile([1, n], f32)
            nc.gpsimd.dma_start(rrow[:], sq[0:1, :])
            nc.gpsimd.dma_start(rrow[:], sq[1:2, :], accum_op=ALU.add)

            # ---- bf16 hi/lo splits (pseudo-fp32 matmul operands) ----
            # hi = bf16(v); lo = bf16(v - hi). Sources: z, s = 0.2 z, mr = -0.1 r.
            zh = singles.tile([2, n], bf16)
            zl = singles.tile([2, n], bf16)
            sh = singles.tile([2, n], bf16)
            sl = singles.tile([2, n], bf16)
            mrh = singles.tile([1, n], bf16)
            mrl = singles.tile([1, n], bf16)
            tmp = singles.tile([2, n], f32)

            nc.scalar.activation(zh[:], zt[:], AF.Copy)
            nc.vector.tensor_tensor(tmp[:], zt[:], zh[:], ALU.subtract)
            nc.vector.tensor_copy(zl[:], tmp[:])
            nc.scalar.activation(sh[:], zt[:], AF.Copy, scale=0.2)
            nc.vector.scalar_tensor_tensor(
                tmp[:], zt[:], 0.2, sh[:], op0=ALU.mult, op1=ALU.subtract
            )
            nc.vector.tensor_copy(sl[:], tmp[:])
            nc.scalar.activation(mrh[:], rrow[:], AF.Copy, scale=-0.1)
            nc.vector.scalar_tensor_tensor(
                tmp[0:1, :], rrow[:], -0.1, mrh[:], op0=ALU.mult, op1=ALU.subtract
            )
            nc.vector.tensor_copy(mrl[:], tmp[0:1, :])

            ones2 = singles.tile([2, n], bf16)
            nc.vector.memset(ones2[:], 1.0)

            # K=10 row pairing (lhsT row k x rhs row k):
            #  k0: 1*mrh_j  k1: 1*mrl_j  k2: mrh_i*1  k3: mrl_i*1
            #  k4: sxh_i*zxh_j  k5: syh_i*zyh_j  k6: sxh_i*zxl_j
            #  k7: syh_i*zyl_j  k8: sxl_i*zxh_j  k9: syl_i*zyh_j
            zcol = singles.tile([10, n], bf16)  # rhs (j side)
            nc.gpsimd.dma_start(zcol[0:1, :], mrh[:])
            nc.gpsimd.dma_start(zcol[1:2, :], mrl[:])
            nc.gpsimd.dma_start(zcol[2:4, :], ones2[:])
            nc.gpsimd.dma_start(zcol[4:6, :], zh[:])
            nc.gpsimd.dma_start(zcol[6:8, :], zl[:])
            nc.gpsimd.dma_start(zcol[8:10, :], zh[:])

            zrow = singles.tile([10, n], bf16)  # lhsT (i side)
            nc.gpsimd.dma_start(zrow[0:2, :], ones2[:])
            nc.gpsimd.dma_start(zrow[2:3, :], mrh[:])
            nc.gpsimd.dma_start(zrow[3:4, :], mrl[:])
            nc.gpsimd.dma_start(zrow[4:6, :], sh[:])
            nc.gpsimd.dma_start(zrow[6:8, :], sh[:])
            nc.gpsimd.dma_start(zrow[8:10, :], sl[:])

            acc = singles.tile([128, nb * nch], f32)

            # ---- main loop: 16 row blocks x nch chunks ----
            for ib in range(nb):
                mk = maskp.tile([128, n], i32)
                nc.sync.dma_start(mk[:], m_in[ib * 128:(ib + 1) * 128, :])
                for h in range(nch):
                    ps = psump.tile([128, cw], f32)
                    for jc in range(nj):
                        c0 = h * cw + jc * 512
                        nc.tensor.matmul(
                            ps[:, jc * 512:(jc + 1) * 512],
                            zrow[:, ib * 128:(ib + 1) * 128],
                            zcol[:, c0:c0 + 512],
                            start=True,
                            stop=True,
                        )
                    nc.scalar.activation(ps[:], ps[:], AF.Exp)
                    nc.vector.scalar_tensor_tensor(
                        ps[:], ps[:], 1.0, mk[:, h * cw:(h + 1) * cw],
                        op0=ALU.subtract, op1=ALU.mult,
                        accum_out=acc[:, ib * nch + h:ib * nch + h + 1],
                    )
            nc.gpsimd.dma_start(out[:], acc[:])
    nc.compile()
    return nc


def _get_graph():
    global _CACHED
    if _CACHED is None:
        _CACHED = _build()
    return _CACHED


def kernel(embedding, abs_coords, patch_mask, _trace=False, _trace_kwargs=None):
    from concourse.bass_utils import run_bass_kernel_spmd

    nc = _get_graph()
    in_maps = [
        {
            "e": np.ascontiguousarray(embedding[b].T, dtype=np.float32),
            "a": np.ascontiguousarray(abs_coords[b].T, dtype=np.float32),
            "m": np.ascontiguousarray(patch_mask[b], dtype=np.int32),
        }
        for b in range(B)
    ]
    kw = {}
    if _trace:
        kw = dict(trace=True, **(_trace_kwargs or {}))
    res = run_bass_kernel_spmd(nc, in_maps, core_ids=list(range(B)), **kw)
    total = -sum(
        float(np.sum(r["out"], dtype=np.float64)) for r in res.results
    )
    out = np.float32(total)
    if _trace:
        return out, res
    return out


# revision 6
# speedup vs baseline: 2.2478x; 1.1876x over previous
"""AnchorLoss Trainium2 kernel.

loss = sum_{b,i,j: mask[b,i,j]==1} (1 - exp(-|z_i - z_j|^2 / 10)),  z = embedding + abs_coords

Sharding: data-parallel over batch B=8, one batch per NeuronCore. Each core:
  - device-side prep: z = e + a, r = |z|^2, bf16 hi/lo splits (pseudo-fp32),
  - streams its [2048, 2048] int32 mask in 16 row-blocks of [128, 2048],
  - per 1024-col chunk: K=10 bf16 matmul -> PSUM = d2 (hi/lo expansion),
    ScalarE exp with scale=-0.1, VectorE fused (E - 1) * mask with
    per-partition accumulate,
  - returns [128, 32] partial sums; host sums and negates.

The host passes e/a stacked+transposed as one [2, 2N] array (layout only,
zero flops) so the coordinate load is a single 2-descriptor DMA.
"""
import numpy as np
import sys

for _p in ("/opt/trn_rl_repo", "/root/.axon_site/_ro/trn_rl_repo"):
    if _p not in sys.path:
        sys.path.append(_p)

N = 2048
B = 8

_CACHED = None


def _build(n=N):
    from concourse import bacc, mybir, tile

    f32 = mybir.dt.float32
    i32 = mybir.dt.int32
    bf16 = mybir.dt.bfloat16
    AF = mybir.ActivationFunctionType
    ALU = mybir.AluOpType

    nb = n // 128          # mask row blocks
    cw = min(n, 1024)      # pipeline chunk width
    nch = n // cw          # chunks per row block
    nj = cw // 512         # matmuls per chunk

    nc = bacc.Bacc()
    ea_in = nc.declare_dram_parameter("ea", [2, 2 * n], f32, isOutput=False)
    m_in = nc.declare_dram_parameter("m", [n, n], i32, isOutput=False)
    out = nc.declare_dram_parameter("out", [128, nb * nch], f32, isOutput=True)

    with tile.TileContext(nc) as tc:
        with (
            tc.tile_pool(name="singles", bufs=1) as singles,
            tc.tile_pool(name="maskp", bufs=4) as maskp,
            tc.tile_pool(name="psum", bufs=4, space="PSUM") as psump,
        ):
            # warm the ACT exp table set off the critical path
            dummy = singles.tile([1, 8], f32)
            nc.gpsimd.memset(dummy[:], 0.0)
            nc.scalar.activation(dummy[:], dummy[:], AF.Exp)

            # ---- prep: zT [2, n], rT [1, n] ----
            ea = singles.tile([2, 2 * n], f32)   # rows: [ex | ax], [ey | ay]
            nc.gpsimd.dma_start(ea[:], ea_in[:])
            zt = singles.tile([2, n], f32)
            nc.vector.tensor_tensor(zt[:], ea[:, 0:n], ea[:, n:2 * n], ALU.add)
            sq = singles.tile([2, n], f32)
            nc.vector.tensor_tensor(sq[:], zt[:], zt[:], ALU.mult)
            rrow = singles.tile([1, n], f32)
            nc.gpsimd.dma_start(rrow[:], sq[0:1, :])
            nc.gpsimd.dma_start(rrow[:], sq[1:2, :], accum_op=ALU.add)

            # ---- bf16 hi/lo splits: hi = bf16(v), lo = bf16(v - hi) ----
            zh = singles.tile([2, n], bf16)
            zl = singles.tile([2, n], bf16)
            rh = singles.tile([1, n], bf16)
            rl = singles.tile([1, n], bf16)
            m2zh = singles.tile([2, n], bf16)   # -2 * zh (exact in bf16)
            m2zl = singles.tile([2, n], bf16)
            nc.scalar.activation(zh[:], zt[:], AF.Copy)
            nc.vector.tensor_tensor(zl[:], zt[:], zh[:], ALU.subtract)
            nc.scalar.activation(rh[:], rrow[:], AF.Copy)
            nc.vector.tensor_tensor(rl[:], rrow[:], rh[:], ALU.subtract)
            nc.vector.tensor_scalar_mul(m2zh[:], zh[:], -2.0)
            nc.vector.tensor_scalar_mul(m2zl[:], zl[:], -2.0)
            ones2 = singles.tile([2, n], bf16)
            nc.vector.memset(ones2[:], 1.0)

            # K=10 row pairing (lhsT row k x rhs row k) -> PSUM = d2:
            #  k0: 1*rh_j   k1: 1*rl_j   k2: rh_i*1   k3: rl_i*1
            #  k4: zxh*m2zxh  k5: zyh*m2zyh  k6: zxh*m2zxl
            #  k7: zyh*m2zyl  k8: zxl*m2zxh  k9: zyl*m2zyh
            zcol = singles.tile([10, n], bf16)  # rhs (j side)
            nc.gpsimd.dma_start(zcol[0:1, :], rh[:])
            nc.gpsimd.dma_start(zcol[1:2, :], rl[:])
            nc.gpsimd.dma_start(zcol[2:4, :], ones2[:])
            nc.gpsimd.dma_start(zcol[4:6, :], m2zh[:])
            nc.gpsimd.dma_start(zcol[6:8, :], m2zl[:])
            nc.gpsimd.dma_start(zcol[8:10, :], m2zh[:])

            zrow = singles.tile([10, n], bf16)  # lhsT (i side)
            nc.gpsimd.dma_start(zrow[0:2, :], ones2[:])
            nc.gpsimd.dma_start(zrow[2:3, :], rh[:])
            nc.gpsimd.dma_start(zrow[3:4, :], rl[:])
            nc.gpsimd.dma_start(zrow[4:6, :], zh[:])
            nc.gpsimd.dma_start(zrow[6:8, :], zh[:])
            nc.gpsimd.dma_start(zrow[8:10, :], zl[:])

            acc = singles.tile([128, nb * nch], f32)

            # ---- main loop: nb row blocks x nch chunks ----
            for ib in range(nb):
                mk = maskp.tile([128, n], i32)
                nc.sync.dma_start(mk[:], m_in[ib * 128:(ib + 1) * 128, :])
                for h in range(nch):
                    ps = psump.tile([128, cw], f32)
                    for jc in range(nj):
                        c0 = h * cw + jc * 512
                        nc.tensor.matmul(
                            ps[:, jc * 512:(jc + 1) * 512],
                            zrow[:, ib * 128:(ib + 1) * 128],
                            zcol[:, c0:c0 + 512],
                            start=True,
                            stop=True,
                        )
                    nc.scalar.activation(ps[:], ps[:], AF.Exp, scale=-0.1)
                    nc.vector.scalar_tensor_tensor(
                        ps[:], ps[:], 1.0, mk[:, h * cw:(h + 1) * cw],
                        op0=ALU.subtract, op1=ALU.mult,
                        accum_out=acc[:, ib * nch + h:ib * nch + h + 1],
                    )
            nc.gpsimd.dma_start(out[:], acc[:])
    nc.compile()
    return nc


def _get_graph():
    global _CACHED
    if _CACHED is None:
        _CACHED = _build()
    return _CACHED


def _pack_ea(e, a, n):
    ea = np.empty((2, 2 * n), dtype=np.float32)
    ea[:, :n] = e.T
    ea[:, n:] = a.T
    return ea


def kernel(embedding, abs_coords, patch_mask, _trace=False, _trace_kwargs=None):
    from concourse.bass_utils import run_bass_kernel_spmd

    nc = _get_graph()
    in_maps = [
        {
            "ea": _pack_ea(embedding[b], abs_coords[b], N),
            "m": np.ascontiguousarray(patch_mask[b], dtype=np.int32),
        }
        for b in range(B)
    ]
    kw = {}
    if _trace:
        kw = dict(trace=True, **(_trace_kwargs or {}))
    res = run_bass_kernel_spmd(nc, in_maps, core_ids=list(range(B)), **kw)
    total = -sum(
        float(np.sum(r["out"], dtype=np.float64)) for r in res.results
    )
    out = np.float32(total)
    if _trace:
        return out, res
    return out
